# revision 1
# baseline (speedup 1.0000x reference)
"""Trainium2 Bass kernel for nn_Event_Critic_Net (dual-branch GAT critic).

Math: the reference only reads the GAT output at the LAST node of each
graph (graphs are 32 contiguous nodes), so only edges whose dst is a
graph's last node contribute.  For those edges the softmax-weighted
aggregation commutes with the linear projection W:

    out_g = sigmoid( (sum_n alpha[n] * x[n,:]) @ W + bias )
    alpha[n] = cnt[n]*exp(e[n]) / (sum_n cnt[n]*exp(e[n]) + 1e-16)
    e[n] = leaky_relu(x[n]. w_src + x[last(g)]. w_dst),  w_* = W @ att_*

cnt[n] = number of edges (n -> last(g(n))).  Graph-structure prep
(edge counts, tiling, transposed copy, weight replication) happens on
host; all FLOPs on device.  Sharding: graphs are data-parallel across
the 8 cores (core c owns graphs [c*512,(c+1)*512)).

x is shipped twice in bf16: node-major (y aggregation, PE contracts
over nodes) and s-major `xt` (attention logits, PE contracts over
features).  PSUM accumulates fp32; softmax scalars stay fp32.
"""

import numpy as np
from contextlib import ExitStack

NC = 8            # cores
N = 131072        # nodes total
G = 4096          # graphs
NPG = 32          # nodes per graph
S = 64            # state size
H = 128           # hidden size
NPC = N // NC     # 16384 nodes per core
GPC = G // NC     # 512 graphs per core
T = NPC // 128    # 128 node-tiles per core
SA = 66           # x columns: 64 features | ones@64 | zero pad
TH = T // 2       # half-branch tiles

_CACHE = {}


def _build_module():
    import concourse.tile as tile
    from concourse import bacc, mybir
    from concourse.alu_op_type import AluOpType as Alu

    f32 = mybir.dt.float32
    bf16 = mybir.dt.bfloat16
    Act = mybir.ActivationFunctionType
    AxX = mybir.AxisListType.X

    nc = bacc.Bacc("TRN2", target_bir_lowering=False, debug=False,
                   num_devices=NC)

    dram = {}

    def din(name, shape, dt=f32):
        dram[name] = nc.dram_tensor(name, shape, dt, kind="ExternalInput")

    for p in ("u", "d"):
        din(f"{p}_xab", [128, T * SA], bf16)
        din(f"{p}_xt", [128, NPC // 2], bf16)
        din(f"{p}_cnt", [128, T])
        din(f"{p}_xlast", [128, 4 * S], bf16)
    din("cstf", [128, 200])
    din("cstb", [128, 912], bf16)
    out_dram = nc.dram_tensor("out", [1, GPC], f32, kind="ExternalOutput")

    with tile.TileContext(nc) as tc, ExitStack() as ctx:
        const = ctx.enter_context(tc.tile_pool(name="const", bufs=1))
        xp = ctx.enter_context(tc.tile_pool(name="xp", bufs=2))
        wk = ctx.enter_context(tc.tile_pool(name="wk", bufs=2))
        ps1 = ctx.enter_context(tc.tile_pool(name="ps1", bufs=1, space="PSUM"))
        ps2 = ctx.enter_context(tc.tile_pool(name="ps2", bufs=2, space="PSUM"))

        cstf = const.tile([128, 200], f32, tag="cstf")
        nc.gpsimd.dma_start(cstf[:], dram["cstf"].ap())
        cstb = const.tile([128, 912], bf16, tag="cstb")
        nc.gpsimd.dma_start(cstb[:], dram["cstb"].ap())
        Bm = cstf[:, 0:4]
        eps = cstf[0:1, 4:5]
        mlpb = cstf[0:1, 5:6]
        biases = {"u": cstf[:, 6:7], "d": cstf[:, 7:8]}
        ones64 = cstf[0:1, 8:8 + S]
        ident = cstf[:, 72:200]
        Qm = cstb[0:4, 0:128]
        wv4s = {"u": cstb[:, 128:132], "d": cstb[:, 132:136]}
        wdsts = {"u": cstb[:, 136:392], "d": cstb[:, 392:648]}
        Ws = {"u": cstb[0:64, 648:776], "d": cstb[0:64, 776:904]}
        mlpW = cstb[:, 904:905]

        sig = {}
        st = {"u": {}, "d": {}}
        # ---- big loads: xt then xab; branch u via Sync DGE, d via Scalar ----
        for p, eng in (("u", nc.sync), ("d", nc.scalar)):
            xt2 = []
            for c in range(2):
                t = xp.tile([128, NPC // 4], bf16, tag=f"xt{c}",
                            name=f"xt{c}_{p}")
                eng.dma_start(
                    t[:], dram[f"{p}_xt"].ap()[:, c * NPC // 4:
                                               (c + 1) * NPC // 4])
                xt2.append(t)
            st[p]["xt"] = xt2
            xq = []
            for c in range(2):
                t = xp.tile([128, TH * SA], bf16, tag=f"x{c}",
                            name=f"x{c}_{p}")
                eng.dma_start(
                    t[:], dram[f"{p}_xab"].ap()[:, c * TH * SA:
                                                (c + 1) * TH * SA])
                xq.append(t)
            st[p]["x"] = xq

        # ---- phase A (both branches): small loads + attention logits ----
        for p in ("u", "d"):
            s = st[p]
            wv4 = wv4s[p]
            wdst = wdsts[p]
            s["Wb"] = Ws[p]
            s["bias"] = biases[p]

            cnt = wk.tile([128, T], f32, tag="cnt", name=f"cnt_{p}")
            s["cnt"] = cnt
            nc.gpsimd.dma_start(cnt[:], dram[f"{p}_cnt"].ap())
            xl = wk.tile([128, 4 * S], bf16, tag="xl")
            nc.gpsimd.dma_start(xl[:], dram[f"{p}_xlast"].ap())
            xt2 = s["xt"]

            # a_src per node on PE: one f=4 matmul covers two node-tiles
            # (chunk c: cols 4c+0/1 = tile c, cols 4c+2/3 = tile 64+c)
            asps = ps2.tile([128, 2 * T], f32, tag="asps", name=f"asps_{p}")
            s["asps"] = asps
            for c in range(T // 2):
                xtc = xt2[c // 32]
                cc = c % 32
                nc.tensor.matmul(
                    asps[0:128, 4 * c:4 * c + 4],
                    xtc[:, 128 * cc:128 * cc + 128],
                    wv4,
                    start=True, stop=True)

            # a_dst at last nodes: mult+reduce, transpose, broadcast
            tmp4 = wk.tile([128, 4 * S], bf16, tag="tmp4")
            nc.vector.tensor_tensor(tmp4[:], xl[:], wdst, op=Alu.mult)
            adst = wk.tile([128, 4], f32, tag="adst")
            nc.vector.tensor_reduce(
                adst[:], tmp4[:].rearrange("p (j s) -> p j s", s=S),
                axis=AxX, op=Alu.add)
            tp = ps1.tile([4, 128], f32, tag="mix")
            nc.tensor.transpose(tp[:], adst[:], ident)
            adT = wk.tile([4, 128], bf16, tag="adT")
            nc.vector.tensor_copy(adT[:], tp[:])
            adbc_ps = ps1.tile([128, T], f32, tag="adbc")
            nc.tensor.matmul(adbc_ps[:], Qm, adT[:], start=True, stop=True)
            adbc = wk.tile([128, T], f32, tag="adbcs", name=f"adbcs_{p}")
            s["adbc"] = adbc
            nc.vector.tensor_copy(adbc[:], adbc_ps[:])

        # ---- phase B (both branches): P/M, aggregation, normalize ----
        for p in ("u", "d"):
            s = st[p]
            x, cnt, adbc, asps = s["x"], s["cnt"], s["adbc"], s["asps"]
            M = wk.tile([128, 4 * T], bf16, tag="M")
            Mv = M[:].rearrange("p (i j) -> p i j", j=4)
            for h in range(2):
                hs = slice(h * TH, (h + 1) * TH)
                asrc = wk.tile([128, TH], f32, tag="asrc")
                nc.vector.tensor_copy(asrc[:], asps[:, 2 * h::4])
                z = wk.tile([128, TH], f32, tag="z")
                nc.vector.tensor_tensor(z[:], asrc[:], adbc[:, hs],
                                        op=Alu.add)
                e = wk.tile([128, TH], f32, tag="e")
                nc.vector.scalar_tensor_tensor(
                    e[:], z[:], 0.2, z[:], op0=Alu.mult, op1=Alu.max)
                ex = wk.tile([128, TH], f32, tag="ex")
                nc.scalar.activation(ex[:], e[:], Act.Exp)
                P = wk.tile([128, TH], f32, tag="P")
                nc.vector.tensor_tensor(P[:], ex[:], cnt[:, hs], op=Alu.mult)
                for j in range(4):
                    nc.vector.tensor_scalar(
                        Mv[:, hs, j], P[:], Bm[:, j:j + 1], None, op0=Alu.mult)

            ynT = ps2.tile([128, 4 * T], f32, tag="ynT")
            for i in range(T):
                xc = x[i // TH]
                ii = i % TH
                nc.tensor.matmul(
                    ynT[0:SA, 4 * i:4 * (i + 1)],
                    xc[:, SA * ii:SA * (ii + 1)],
                    M[:, 4 * i:4 * (i + 1)],
                    start=True, stop=True)

            # normalize by denominator (row 64 of y^T)
            ysb = wk.tile([S + 1, GPC], f32, tag="ysb")
            nc.vector.tensor_copy(ysb[:], ynT[0:S + 1, :])
            dn = wk.tile([1, GPC], f32, tag="dn")
            nc.vector.tensor_scalar(
                dn[:], ysb[S:S + 1, :], eps, None, op0=Alu.add)
            rp = wk.tile([1, GPC], f32, tag="rp")
            nc.vector.reciprocal_approx_fast(rp[:], dn[:])
            rbc = ps1.tile([S, GPC], f32, tag="mix")
            nc.tensor.matmul(rbc[:], ones64, rp[:], start=True, stop=True)
            ynrm = wk.tile([S, GPC], bf16, tag="ynrm")
            nc.vector.tensor_tensor(ynrm[:], ysb[0:S, :], rbc[:], op=Alu.mult)

            # project + bias + sigmoid
            hT = ps1.tile([H, GPC], f32, tag="hT")
            nc.tensor.matmul(hT[:], s["Wb"], ynrm[:], start=True, stop=True)
            sg = wk.tile([H, GPC], bf16, tag="sig")
            nc.scalar.activation(sg[:], hT[:], Act.Sigmoid, bias=s["bias"])
            sig[p] = sg

        # ---- combine branches + MLP head ----
        prod = wk.tile([H, GPC], bf16, tag="prod")
        nc.vector.tensor_tensor(prod[:], sig["u"][:], sig["d"][:], op=Alu.mult)
        o_ps = ps1.tile([1, GPC], f32, tag="mix")
        nc.tensor.matmul(o_ps[:], mlpW, prod[:], start=True, stop=True)
        o_sb = wk.tile([1, GPC], f32, tag="o_sb")
        nc.vector.tensor_scalar(
            o_sb[:], o_ps[:], mlpb, None, op0=Alu.add)
        nc.sync.dma_start(out_dram.ap(), o_sb[:])

    nc.compile()
    return nc


def _get_module():
    if "nc" not in _CACHE:
        _CACHE["nc"] = _build_module()
    return _CACHE["nc"]


def _prep_branch(x, ei, W, att_src, att_dst, bias):
    """Host-side sharding + graph-format prep for one branch."""
    import ml_dtypes
    bf = ml_dtypes.bfloat16
    x = np.asarray(x, np.float32)
    src = np.asarray(ei[0]).astype(np.int64)
    dst = np.asarray(ei[1]).astype(np.int64)
    W = np.asarray(W, np.float32)
    w_src = (W @ np.asarray(att_src, np.float32)).astype(np.float32)
    w_dst = (W @ np.asarray(att_dst, np.float32)).astype(np.float32)

    valid = (dst % NPG) == (NPG - 1)
    cnt = np.bincount(src[valid], minlength=N).astype(np.float32)

    per_core = []
    for c in range(NC):
        xs = x[c * NPC:(c + 1) * NPC]
        xab = np.zeros((T, 128, SA), np.float32)
        xab[:, :, :S] = xs.reshape(T, 128, S)
        xab[:, :, S] = 1.0
        xab = np.ascontiguousarray(
            xab.transpose(1, 0, 2).reshape(128, T * SA)).astype(bf)
        # xt[64k+s, m] = x[8192k + m, s]
        xtv = xs.reshape(2, NPC // 2, S).transpose(0, 2, 1)
        xtv = np.ascontiguousarray(xtv.reshape(128, NPC // 2)).astype(bf)
        cnt_t = np.ascontiguousarray(
            cnt[c * NPC:(c + 1) * NPC].reshape(T, 128).T)
        xlast = np.ascontiguousarray(
            xs[NPG - 1::NPG].reshape(128, 4 * S)).astype(bf)
        per_core.append({"xab": xab, "xt": xtv, "cnt": cnt_t, "xlast": xlast})

    wv4 = np.zeros((128, 4), np.float32)
    wv4[:S, 0] = w_src
    wv4[:S, 1] = w_dst
    wv4[S:, 2] = w_src
    wv4[S:, 3] = w_dst
    wdst_rep = np.broadcast_to(w_dst, (128, 4, S)).reshape(128, 4 * S)
    shared = {
        "wv4": wv4.astype(np.float32),
        "wdst": wdst_rep.astype(np.float32),
        "W": W,
        "bias": np.asarray(bias, np.float32).reshape(H, 1),
    }
    return per_core, shared


def _build_in_maps(inputs):
    import ml_dtypes
    bf = ml_dtypes.bfloat16
    pcs = {}
    shareds = {}
    pcs["u"], shareds["u"] = _prep_branch(
        inputs["up_x"], inputs["up_edge_index"], inputs["up_W"],
        inputs["up_att_src"], inputs["up_att_dst"], inputs["up_bias"])
    pcs["d"], shareds["d"] = _prep_branch(
        inputs["down_x"], inputs["down_edge_index"], inputs["down_W"],
        inputs["down_att_src"], inputs["down_att_dst"], inputs["down_bias"])

    pp = np.arange(128)
    cstf = np.zeros((128, 200), np.float32)
    cstf[pp, pp // 32] = 1.0                       # Bm cols 0:4
    cstf[0, 4] = 1e-16                             # eps
    cstf[0, 5] = float(np.asarray(inputs["mlp_b"]).reshape(-1)[0])
    cstf[:, 6] = shareds["u"]["bias"][:, 0]
    cstf[:, 7] = shareds["d"]["bias"][:, 0]
    cstf[0, 8:8 + S] = 1.0                         # ones64
    cstf[:, 72:200] = np.eye(128, dtype=np.float32)

    cstb = np.zeros((128, 912), np.float32)
    cstb[pp // 32, pp] = 0.0
    Qm = np.zeros((4, 128), np.float32)
    Qm[np.arange(128) // 32, np.arange(128)] = 1.0
    cstb[0:4, 0:128] = Qm
    cstb[:, 128:132] = shareds["u"]["wv4"]
    cstb[:, 132:136] = shareds["d"]["wv4"]
    cstb[:, 136:392] = shareds["u"]["wdst"]
    cstb[:, 392:648] = shareds["d"]["wdst"]
    cstb[0:64, 648:776] = shareds["u"]["W"]
    cstb[0:64, 776:904] = shareds["d"]["W"]
    cstb[:, 904] = np.asarray(inputs["mlp_W"], np.float32).reshape(H)

    common = {
        "cstf": cstf,
        "cstb": cstb.astype(bf),
    }

    in_maps = []
    for c in range(NC):
        m = dict(common)
        for p in ("u", "d"):
            for k, v in pcs[p][c].items():
                m[f"{p}_{k}"] = v
        in_maps.append(m)
    return in_maps


def kernel(**inputs):
    from concourse.bass_utils import run_bass_kernel_spmd

    nc = _get_module()
    in_maps = _build_in_maps(inputs)
    res = run_bass_kernel_spmd(nc, in_maps, core_ids=list(range(NC)))
    out = np.concatenate(
        [np.asarray(r["out"], np.float32).reshape(GPC) for r in res.results])
    return out.reshape(G, 1)



# revision 5
# speedup vs baseline: 1.1297x; 1.1297x over previous
"""Trainium2 Bass kernel for nn_Event_Critic_Net (dual-branch GAT critic).

Math: the reference only reads the GAT output at the LAST node of each
graph (graphs are 32 contiguous nodes), so only edges whose dst is a
graph's last node contribute.  For those edges the softmax-weighted
aggregation commutes with the linear projection W:

    out_g = sigmoid( (sum_n alpha[n] * x[n,:]) @ W + bias )
    alpha[n] = cnt[n]*exp(e[n]) / (sum_n cnt[n]*exp(e[n]) + 1e-16)
    e[n] = leaky_relu(x[n]. w_src + x[last(g)]. w_dst),  w_* = W @ att_*

cnt[n] = number of edges (n -> last(g(n))).  Per graph only ~7 distinct
source nodes have cnt>0, so the host COMPACTS each graph to K=20 node
slots (zero-padded); 6 graphs share a 128-partition tile (120 rows used)
-> T=86 tiles per core instead of 128.  Graph-structure prep (edge
counts, gather, tiling, transposed copy, weight replication) happens on
host; all FLOPs on device.  Sharding: graphs are data-parallel across
the 8 cores (core c owns graphs [c*512,(c+1)*512)).

Device pipeline per branch (all matmul stationaries bf16):
  logits : xt-chunk [128,128] stationary (FWL), wv2 [128,2] moving
           -> asps psum [128, 2*43] (node-layout, 1 bank)
  a_dst  : xl2 mult+reduce -> transpose -> Qm6 matmul -> adbc [128,86]
  P-chain: z=asps+adbc, e=leakyrelu, exp (ACT), P=e*cnt   [128,86]
  M-build: one tensor_tensor with to_broadcast: M[p,(t,j)]=P[p,t]*Bm6[p,j]
  agg    : per tile t: xg-tile [128,66] stationary, M[:,6t:6t+6] moving
           -> ynT psum [66, 258] x2; row 64 = denominator (ones column)
  norm   : recip(denom) -> rbc = ones64 (x) recip (matmul) -> ynrm
  proj   : Wb [64,128] stationary, ynrm [64,512] moving -> h psum
           sigmoid+bias via ACT directly off psum
  tail   : prod = sg_u*sg_d, mlp matmul -> [1,512] + b -> DMA out
"""

import numpy as np
from contextlib import ExitStack

NC = 8            # cores
N = 131072        # nodes total
G = 4096          # graphs
NPG = 32          # nodes per graph
S = 64            # state size
H = 128           # hidden size
GPC = G // NC     # 512 graphs per core
K = 20            # node slots per graph (max distinct srcs observed: 18)
GPT = 6           # graphs per tile (6*20=120 rows used of 128)
T = 86            # ceil(512/6) tiles per core
SA = 66           # xg tile cols: 64 feats | ones | pad
NT = T * 128      # 11008 slot-rows per core per branch
XTC = NT // 2     # 5504 xt columns
NCH = XTC // 128  # 43 logit chunks

_CACHE = {}


def _build_module():
    import concourse.tile as tile
    from concourse import bacc, mybir
    from concourse.alu_op_type import AluOpType as Alu

    f32 = mybir.dt.float32
    bf16 = mybir.dt.bfloat16
    Act = mybir.ActivationFunctionType
    AxX = mybir.AxisListType.X

    nc = bacc.Bacc("TRN2", target_bir_lowering=False, debug=False,
                   num_devices=NC)

    dram = {}

    def din(name, shape, dt=f32):
        dram[name] = nc.dram_tensor(name, shape, dt, kind="ExternalInput")

    for p in ("u", "d"):
        din(f"{p}_xg", [128, T * SA], bf16)
        din(f"{p}_xt", [128, XTC], bf16)
        din(f"{p}_cnt", [128, T])
        din(f"{p}_xl2", [128, GPT * S], bf16)
    din("cstf", [128, 202])
    din("cstb", [128, 520], bf16)
    out_dram = nc.dram_tensor("out", [1, GPC], f32, kind="ExternalOutput")

    # chunked loads: xt split at 128-col multiples, xg at SA-col multiples
    XT_SPLIT = [0, 11, 22, 33, NCH]      # x128 cols
    XG_SPLIT = [0, 22, 44, 65, T]        # xSA cols

    with tile.TileContext(nc) as tc, ExitStack() as ctx:
        const = ctx.enter_context(tc.tile_pool(name="const", bufs=1))
        xp = ctx.enter_context(tc.tile_pool(name="xp", bufs=2))
        wk = ctx.enter_context(tc.tile_pool(name="wk", bufs=2))
        ps1 = ctx.enter_context(tc.tile_pool(name="ps1", bufs=1, space="PSUM"))
        psA = ctx.enter_context(tc.tile_pool(name="psA", bufs=2, space="PSUM"))
        psY = ctx.enter_context(tc.tile_pool(name="psY", bufs=2, space="PSUM"))

        cstf = const.tile([128, 202], f32, tag="cstf")
        nc.gpsimd.dma_start(cstf[:], dram["cstf"].ap())
        cstb = const.tile([128, 520], bf16, tag="cstb")
        nc.gpsimd.dma_start(cstb[:], dram["cstb"].ap())
        ident = cstf[:, 0:128]
        eps = cstf[0:1, 128:129]
        mlpb = cstf[0:1, 129:130]
        biases = {"u": cstf[:, 130:131], "d": cstf[:, 131:132]}
        Bm6f = cstf[:, 132:138]
        ones64 = cstf[0:1, 138:202]
        wv2s = {"u": cstb[:, 0:2], "d": cstb[:, 2:4]}
        wdsts = {"u": cstb[:, 4:68], "d": cstb[:, 68:132]}
        Qm6 = cstb[0:GPT, 132:260]
        Ws = {"u": cstb[0:S, 260:388], "d": cstb[0:S, 388:516]}
        mlpW = cstb[:, 516:517]

        st = {"u": {}, "d": {}}
        # ---- big loads: branch u via Sync DGE, d via Scalar DGE ----
        for p, eng in (("u", nc.sync), ("d", nc.scalar)):
            xts = []
            for i in range(4):
                w = (XT_SPLIT[i + 1] - XT_SPLIT[i]) * 128
                t_ = xp.tile([128, w], bf16, tag=f"xt{i}", name=f"xt{i}_{p}")
                eng.dma_start(t_[:], dram[f"{p}_xt"].ap()[
                    :, XT_SPLIT[i] * 128:XT_SPLIT[i + 1] * 128])
                xts.append(t_)
            st[p]["xt"] = xts
            xgs = []
            for i in range(4):
                w = (XG_SPLIT[i + 1] - XG_SPLIT[i]) * SA
                t_ = xp.tile([128, w], bf16, tag=f"xg{i}", name=f"xg{i}_{p}")
                eng.dma_start(t_[:], dram[f"{p}_xg"].ap()[
                    :, XG_SPLIT[i] * SA:XG_SPLIT[i + 1] * SA])
                xgs.append(t_)
            st[p]["xg"] = xgs

        # ---- per-branch compute ----
        for p in ("u", "d"):
            s = st[p]
            cnt = wk.tile([128, T], f32, tag="cnt", name=f"cnt_{p}")
            nc.gpsimd.dma_start(cnt[:], dram[f"{p}_cnt"].ap())
            xl2 = wk.tile([128, GPT * S], bf16, tag="xl2", name=f"xl2_{p}")
            nc.gpsimd.dma_start(xl2[:], dram[f"{p}_xl2"].ap())

            # attention logits a_src per slot-row (x-as-weights, FWL chunks)
            # asps[p, 2c+j] = a_src[slot-row 5504j + 128c + p]
            asps = psA.tile([128, 2 * NCH], f32, tag="asps", name=f"as_{p}")
            for c in range(NCH):
                blk = 0
                while XT_SPLIT[blk + 1] <= c:
                    blk += 1
                cc = c - XT_SPLIT[blk]
                nc.tensor.matmul(
                    asps[:, 2 * c:2 * c + 2],
                    s["xt"][blk][:, 128 * cc:128 * cc + 128],
                    wv2s[p],
                    start=True, stop=True)

            # a_dst at last nodes -> adbc [128, T] broadcast per slot
            tmp6 = wk.tile([128, GPT * S], f32, tag="tmp6")
            nc.vector.tensor_tensor(
                tmp6[:].rearrange("p (j s) -> p j s", s=S),
                xl2[:].rearrange("p (j s) -> p j s", s=S),
                wdsts[p].unsqueeze(1).to_broadcast((128, GPT, S)),
                op=Alu.mult)
            adst = wk.tile([128, GPT], f32, tag="adst")
            nc.vector.tensor_reduce(
                adst[:], tmp6[:].rearrange("p (j s) -> p j s", s=S),
                axis=AxX, op=Alu.add)
            tp = ps1.tile([GPT, 128], f32, tag="mix", name=f"adT_{p}")
            nc.tensor.transpose(tp[:], adst[:], ident)
            adT = wk.tile([GPT, 128], bf16, tag="adTs")
            nc.vector.tensor_copy(adT[:], tp[:])
            adbc_ps = ps1.tile([128, T], f32, tag="mix", name=f"adbc_{p}")
            nc.tensor.matmul(adbc_ps[:], Qm6, adT[:, 0:T],
                             start=True, stop=True)
            adbc = wk.tile([128, T], f32, tag="adbcs")
            nc.vector.tensor_copy(adbc[:], adbc_ps[:])

            # P-chain in node-layout [128, T]
            z = wk.tile([128, T], f32, tag="z")
            nc.vector.tensor_tensor(
                z[:].rearrange("p (j c) -> p j c", j=2),
                asps[:].rearrange("p (c j) -> p j c", j=2),
                adbc[:].rearrange("p (j c) -> p j c", j=2), op=Alu.add)
            e = wk.tile([128, T], f32, tag="e")
            nc.vector.scalar_tensor_tensor(
                e[:], z[:], 0.2, z[:], op0=Alu.mult, op1=Alu.max)
            ex = wk.tile([128, T], f32, tag="ex")
            nc.scalar.activation(ex[:], e[:], Act.Exp)
            P = wk.tile([128, T], f32, tag="P")
            nc.vector.tensor_tensor(P[:], ex[:], cnt[:], op=Alu.mult)

            # M[p, (t,j)] = P[p,t] * Bm6[p,j]
            M = wk.tile([128, T * GPT], bf16, tag="M")
            nc.vector.tensor_tensor(
                M[:].rearrange("p (t j) -> p t j", j=GPT),
                P[:].unsqueeze(2).to_broadcast((128, T, GPT)),
                Bm6f.unsqueeze(1).to_broadcast((128, T, GPT)),
                op=Alu.mult)

            # aggregation: ynT[s, (t,j)] += x[n, s] * M[n, (t,j)]
            TH2 = T // 2
            yns = wk.tile([SA, T * GPT], f32, tag="yns", name=f"yns_{p}")
            for h in range(2):
                ynT = psY.tile([SA, TH2 * GPT], f32, tag="ynT",
                               name=f"ynT_{p}{h}")
                for i in range(TH2):
                    t = h * TH2 + i
                    blk = 0
                    while XG_SPLIT[blk + 1] <= t:
                        blk += 1
                    tt = t - XG_SPLIT[blk]
                    nc.tensor.matmul(
                        ynT[:, GPT * i:GPT * (i + 1)],
                        s["xg"][blk][:, SA * tt:SA * tt + SA],
                        M[:, GPT * t:GPT * (t + 1)],
                        start=True, stop=True)
                nc.vector.tensor_copy(
                    yns[:, h * TH2 * GPT:(h + 1) * TH2 * GPT], ynT[:])

            # normalize (graphs 0:512 only; cols 512:515 are padding)
            dn = wk.tile([1, GPC], f32, tag="dn")
            nc.vector.tensor_scalar(
                dn[:], yns[S:S + 1, 0:GPC], eps, None, op0=Alu.add)
            rp = wk.tile([1, GPC], f32, tag="rp")
            nc.vector.reciprocal_approx_fast(rp[:], dn[:])
            rbc = ps1.tile([S, GPC], f32, tag="mix", name=f"rbc_{p}")
            nc.tensor.matmul(rbc[:], ones64, rp[:], start=True, stop=True)
            ynrm = wk.tile([S, GPC], bf16, tag="ynrm")
            nc.vector.tensor_tensor(ynrm[:], yns[0:S, 0:GPC], rbc[:],
                                    op=Alu.mult)

            # project + bias + sigmoid (ACT reads psum directly)
            hT = ps1.tile([H, GPC], f32, tag="hT", name=f"hT_{p}")
            nc.tensor.matmul(hT[:], Ws[p], ynrm[:], start=True, stop=True)
            sg = wk.tile([H, GPC], bf16, tag="sig", name=f"sig_{p}")
            nc.scalar.activation(sg[:], hT[:], Act.Sigmoid, bias=biases[p])
            s["sg"] = sg

        # ---- combine branches + MLP head ----
        prod = wk.tile([H, GPC], bf16, tag="prod")
        nc.vector.tensor_tensor(prod[:], st["u"]["sg"][:],
                                st["d"]["sg"][:], op=Alu.mult)
        o_ps = ps1.tile([1, GPC], f32, tag="mix", name="o_ps")
        nc.tensor.matmul(o_ps[:], mlpW, prod[:], start=True, stop=True)
        o_sb = wk.tile([1, GPC], f32, tag="o_sb")
        nc.vector.tensor_scalar(
            o_sb[:], o_ps[:], mlpb, None, op0=Alu.add)
        nc.sync.dma_start(out_dram.ap(), o_sb[:])

    nc.compile()
    return nc


def _get_module():
    if "nc" not in _CACHE:
        _CACHE["nc"] = _build_module()
    return _CACHE["nc"]


def _prep_branch(x, ei, W, att_src, att_dst):
    """Host-side sharding + graph-compaction prep for one branch."""
    import ml_dtypes
    bf = ml_dtypes.bfloat16
    x = np.asarray(x, np.float32)
    src = np.asarray(ei[0]).astype(np.int64)
    dst = np.asarray(ei[1]).astype(np.int64)
    W = np.asarray(W, np.float32)
    w_src = (W @ np.asarray(att_src, np.float32)).astype(np.float32)
    w_dst = (W @ np.asarray(att_dst, np.float32)).astype(np.float32)

    valid = (dst % NPG) == (NPG - 1)
    nodes, counts = np.unique(src[valid], return_counts=True)
    gids = nodes // NPG
    # slot index = rank of node within its graph (nodes sorted by id)
    order = np.argsort(gids, kind="stable")
    gs = gids[order]
    first = np.r_[True, gs[1:] != gs[:-1]]
    idx_of_first = np.maximum.accumulate(
        np.where(first, np.arange(len(gs)), 0))
    slot_sorted = np.arange(len(gs)) - idx_of_first
    slot = np.empty(len(nodes), np.int64)
    slot[order] = slot_sorted
    if slot.size and slot.max() >= K:
        raise ValueError(f"graph has more than K={K} contributing nodes: "
                         f"{slot.max() + 1}")

    per_core = []
    for c in range(NC):
        g_lo, g_hi = c * GPC, (c + 1) * GPC
        m = (gids >= g_lo) & (gids < g_hi)
        nl, cl, gl, sl = nodes[m], counts[m], gids[m] - g_lo, slot[m]
        t = gl // GPT
        part = (gl % GPT) * K + sl
        # xg: [128, T, SA]; rows (part), cols (t, 0:64)=x, col 64 = 1
        xg = np.zeros((128, T, SA), np.float32)
        xg[part, t, :S] = x[nl]
        xg[:, :, S] = 1.0
        xg2 = np.ascontiguousarray(xg.reshape(128, T * SA)).astype(bf)
        cnt_t = np.zeros((128, T), np.float32)
        cnt_t[part, t] = cl.astype(np.float32)
        # xt: [128, XTC]: col m rows 0:64 = feats of slot-row m,
        # rows 64:128 = feats of slot-row XTC+m.  slot-row r = t*128+part
        xflat = np.zeros((NT, S), np.float32)
        xflat[t * 128 + part] = x[nl]
        xtv = xflat.reshape(2, XTC, S).transpose(0, 2, 1)
        xtv = np.ascontiguousarray(xtv.reshape(128, XTC)).astype(bf)
        # xl2[p, j*64:(j+1)*64] = x[last node of graph 6p+j]
        lg = np.arange(g_lo * NPG + NPG - 1, g_hi * NPG, NPG)
        xl = x[lg].reshape(GPC, S)
        xl2 = np.zeros((128, GPT, S), np.float32)
        gg = np.arange(GPC)
        xl2[gg // GPT, gg % GPT] = xl
        xl2 = np.ascontiguousarray(xl2.reshape(128, GPT * S)).astype(bf)
        per_core.append({"xg": xg2, "xt": xtv, "cnt": cnt_t, "xl2": xl2})

    shared = {"w_src": w_src, "w_dst": w_dst, "W": W}
    return per_core, shared


def _build_in_maps(inputs):
    import ml_dtypes
    bf = ml_dtypes.bfloat16
    pcs = {}
    shareds = {}
    pcs["u"], shareds["u"] = _prep_branch(
        inputs["up_x"], inputs["up_edge_index"], inputs["up_W"],
        inputs["up_att_src"], inputs["up_att_dst"])
    pcs["d"], shareds["d"] = _prep_branch(
        inputs["down_x"], inputs["down_edge_index"], inputs["down_W"],
        inputs["down_att_src"], inputs["down_att_dst"])

    pp = np.arange(128)
    cstf = np.zeros((128, 202), np.float32)
    cstf[:, 0:128] = np.eye(128, dtype=np.float32)
    cstf[0, 128] = 1e-16
    cstf[0, 129] = float(np.asarray(inputs["mlp_b"]).reshape(-1)[0])
    cstf[:, 130] = np.asarray(inputs["up_bias"], np.float32)
    cstf[:, 131] = np.asarray(inputs["down_bias"], np.float32)
    # Bm6f [128, 6]: 1 if p//K == j (p < GPT*K)
    cstf[pp[:GPT * K], 132 + pp[:GPT * K] // K] = 1.0
    cstf[0, 138:202] = 1.0

    cstb = np.zeros((128, 520), np.float32)
    for i, p in enumerate(("u", "d")):
        ws = shareds[p]["w_src"]
        cstb[0:S, 0 + 2 * i] = ws
        cstb[S:128, 1 + 2 * i] = ws
        cstb[:, 4 + S * i:4 + S * (i + 1)] = np.broadcast_to(
            shareds[p]["w_dst"], (128, S))
    # Qm6 [6, 128]: 1 if m//K == j (m < GPT*K)
    for j in range(GPT):
        cstb[j, 132 + j * K:132 + (j + 1) * K] = 1.0
    cstb[0:S, 260:388] = shareds["u"]["W"]
    cstb[0:S, 388:516] = shareds["d"]["W"]
    cstb[:, 516] = np.asarray(inputs["mlp_W"], np.float32).reshape(H)

    common = {
        "cstf": cstf,
        "cstb": cstb.astype(bf),
    }

    in_maps = []
    for c in range(NC):
        m = dict(common)
        for p in ("u", "d"):
            for k2, v in pcs[p][c].items():
                m[f"{p}_{k2}"] = v
        in_maps.append(m)
    return in_maps


def kernel(**inputs):
    from concourse.bass_utils import run_bass_kernel_spmd

    nc = _get_module()
    in_maps = _build_in_maps(inputs)
    res = run_bass_kernel_spmd(nc, in_maps, core_ids=list(range(NC)))
    out = np.concatenate(
        [np.asarray(r["out"], np.float32).reshape(GPC) for r in res.results])
    return out.reshape(G, 1)


# revision 8
# speedup vs baseline: 1.4804x; 1.3105x over previous
"""Trainium2 Bass kernel for nn_Event_Critic_Net (dual-branch GAT critic).

Math: the reference only reads the GAT output at the LAST node of each
graph (graphs are 32 contiguous nodes), so only edges whose dst is a
graph's last node contribute.  For those edges the softmax-weighted
aggregation commutes with the linear projection W:

    out_g = sigmoid( (sum_n alpha[n] * x[n,:]) @ W + bias )
    alpha[n] = cnt[n]*exp(e[n]) / (sum_n cnt[n]*exp(e[n]) + 1e-16)
    e[n] = leaky_relu(x[n]. w_src + x[last(g)]. w_dst),  w_* = W @ att_*

cnt[n] = number of edges (n -> last(g(n))).  Per graph only ~7 distinct
source nodes have cnt>0, so the host COMPACTS each graph to K node slots
(zero-padded); GPT graphs share a 128-partition tile -> T tiles per core
instead of 128.  Graph-structure prep (edge counts, gather, tiling,
transposed copy, weight replication) happens on host; all FLOPs on
device.  Sharding: graphs are data-parallel across the 8 cores.

Device pipeline (phases interleaved across branches to keep PE hot):
  logits : xt-chunk [128,128] stationary (FWL), wv2 [128,2] moving
           -> asps psum [128, 2*NCH] (node-layout, 1 bank)
  a_dst  : xl2 mult+reduce -> transpose -> Qm matmul -> adbc [128,T]
  P-chain: z=asps+adbc, e=leakyrelu(DVE), exp(ACT set0), P=e*cnt
  M-build: one tensor_tensor with to_broadcast: M[p,(t,j)]=P[p,t]*Bm[p,j]
  agg    : per tile t: xg-tile [128,66] stationary, M[:,GPT*t..] moving
           -> ynT psum x2; row 64 = denominator (ones column)
  norm   : ACT-Copy evac, recip(DVE) -> rbc = ones64 (x) recip (matmul)
  proj   : ynrm = y*rbc, Wb [64,128] stationary -> h psum [128,512]
  sigmoid: via exp (set0): eu=exp(-h-b); sg_u*sg_d = 1/((1+eu)(1+ed))
  tail   : q=(1+eu)(1+ed), r=recip(q), mlp matmul -> [1,512]+b -> out
"""

import numpy as np
from contextlib import ExitStack

NC = 8            # cores
N = 131072        # nodes total
G = 4096          # graphs
NPG = 32          # nodes per graph
S = 64            # state size
H = 128           # hidden size
GPC = G // NC     # 512 graphs per core
SA = 66           # xg tile cols: 64 feats | ones | pad

_CACHE = {}


def _layout(K):
    GPT = 128 // K               # graphs per tile
    T = -(-GPC // GPT)           # tiles per core
    NT = T * 128                 # slot-rows per core per branch
    XTC = NT // 2                # xt columns
    NCH = XTC // 128             # logit chunks (NT divisible by 256)
    assert NCH * 128 == XTC
    return GPT, T, NT, XTC, NCH


def _build_module(K):
    import concourse.tile as tile
    from concourse import bacc, mybir
    from concourse.alu_op_type import AluOpType as Alu

    GPT, T, NT, XTC, NCH = _layout(K)
    f32 = mybir.dt.float32
    bf16 = mybir.dt.bfloat16
    Act = mybir.ActivationFunctionType
    AxX = mybir.AxisListType.X

    nc = bacc.Bacc("TRN2", target_bir_lowering=False, debug=False,
                   num_devices=NC)

    dram = {}

    def din(name, shape, dt=f32):
        dram[name] = nc.dram_tensor(name, shape, dt, kind="ExternalInput")

    for p in ("u", "d"):
        din(f"{p}_xg", [128, T * SA], bf16)
        din(f"{p}_xt", [128, XTC], bf16)
        din(f"{p}_cnt", [128, T])
        din(f"{p}_xl2", [128, GPT * S], bf16)
    din("cstf", [128, 204])
    din("cstb", [128, 520], bf16)
    out_dram = nc.dram_tensor("out", [1, GPC], f32, kind="ExternalOutput")

    # chunked loads: xt split at 128-col multiples, xg at SA-col multiples
    def split5(n):
        a = max(1, n // 12)
        b = (n - a) // 4
        return [0, a, a + b, a + 2 * b, a + 3 * b, n]
    XT_SPLIT = split5(NCH)
    XG_SPLIT = split5(T)

    with tile.TileContext(nc) as tc, ExitStack() as ctx:
        const = ctx.enter_context(tc.tile_pool(name="const", bufs=1))
        xp = ctx.enter_context(tc.tile_pool(name="xp", bufs=2))
        wk = ctx.enter_context(tc.tile_pool(name="wk", bufs=2))
        ps1 = ctx.enter_context(tc.tile_pool(name="ps1", bufs=1, space="PSUM"))
        psA = ctx.enter_context(tc.tile_pool(name="psA", bufs=2, space="PSUM"))
        psY = ctx.enter_context(tc.tile_pool(name="psY", bufs=2, space="PSUM"))

        cstf = const.tile([128, 204], f32, tag="cstf")
        nc.gpsimd.dma_start(cstf[:], dram["cstf"].ap())
        cstb = const.tile([128, 520], bf16, tag="cstb")
        nc.gpsimd.dma_start(cstb[:], dram["cstb"].ap())
        ident = cstf[:, 0:128]
        eps = cstf[0:1, 128:129]
        mlpb = cstf[0:1, 129:130]
        nbias = {"u": cstf[:, 130:131], "d": cstf[:, 131:132]}  # negated
        Bmf = cstf[:, 132:132 + GPT]
        ones64 = cstf[0:1, 140:204]
        wv2s = {"u": cstb[:, 0:2], "d": cstb[:, 2:4]}
        wdsts = {"u": cstb[:, 4:68], "d": cstb[:, 68:132]}
        Qm = cstb[0:GPT, 132:260]
        Ws = {"u": cstb[0:S, 260:388], "d": cstb[0:S, 388:516]}
        mlpW = cstb[:, 516:517]

        st = {"u": {}, "d": {}}
        # ---- big loads: branch u via Sync DGE, d via Vector DGE ----
        for p, eng in (("u", nc.sync), ("d", nc.scalar)):
            s = st[p]
            s["xt"] = []
            for i in range(5):
                w = (XT_SPLIT[i + 1] - XT_SPLIT[i]) * 128
                t_ = xp.tile([128, w], bf16, tag=f"xt{i}", name=f"xt{i}_{p}")
                eng.dma_start(t_[:], dram[f"{p}_xt"].ap()[
                    :, XT_SPLIT[i] * 128:XT_SPLIT[i + 1] * 128])
                s["xt"].append(t_)
            s["xg"] = []
            for i in range(5):
                w = (XG_SPLIT[i + 1] - XG_SPLIT[i]) * SA
                t_ = xp.tile([128, w], bf16, tag=f"xg{i}", name=f"xg{i}_{p}")
                eng.dma_start(t_[:], dram[f"{p}_xg"].ap()[
                    :, XG_SPLIT[i] * SA:XG_SPLIT[i + 1] * SA])
                s["xg"].append(t_)

        for p in ("u", "d"):
            s = st[p]
            cnt = wk.tile([128, T], f32, tag=f"cnt_{p}")
            nc.gpsimd.dma_start(cnt[:], dram[f"{p}_cnt"].ap())
            s["cnt"] = cnt
            xl2 = wk.tile([128, GPT * S], bf16, tag=f"xl2_{p}")
            nc.gpsimd.dma_start(xl2[:], dram[f"{p}_xl2"].ap())
            s["xl2"] = xl2

        # ---- logits (both branches back to back on PE) ----
        for p in ("u", "d"):
            s = st[p]
            asps = psA.tile([128, 2 * NCH], f32, tag="asps", name=f"as_{p}")
            s["asps"] = asps
            for c in range(NCH):
                blk = 0
                while XT_SPLIT[blk + 1] <= c:
                    blk += 1
                cc = c - XT_SPLIT[blk]
                nc.tensor.matmul(
                    asps[:, 2 * c:2 * c + 2],
                    s["xt"][blk][:, 128 * cc:128 * cc + 128],
                    wv2s[p],
                    start=True, stop=True)

        # ---- a_dst chain + P-chain per branch (DVE/ACT overlap PE) ----
        for p in ("u", "d"):
            s = st[p]
            tmp6 = wk.tile([128, GPT * S], f32, tag=f"tmp6_{p}")
            nc.vector.tensor_tensor(
                tmp6[:].rearrange("p (j s) -> p j s", s=S),
                s["xl2"][:].rearrange("p (j s) -> p j s", s=S),
                wdsts[p].unsqueeze(1).to_broadcast((128, GPT, S)),
                op=Alu.mult)
            adst = wk.tile([128, GPT], f32, tag=f"adst_{p}")
            nc.vector.tensor_reduce(
                adst[:], tmp6[:].rearrange("p (j s) -> p j s", s=S),
                axis=AxX, op=Alu.add)
            tp = ps1.tile([GPT, 128], f32, tag="mix", name=f"adT_{p}")
            nc.tensor.transpose(tp[:], adst[:], ident)
            adT = wk.tile([GPT, 128], bf16, tag=f"adTs_{p}")
            nc.vector.tensor_copy(adT[:], tp[:])
            adbc_ps = ps1.tile([128, T], f32, tag="mix", name=f"adbc_{p}")
            nc.tensor.matmul(adbc_ps[:], Qm, adT[:, 0:T],
                             start=True, stop=True)
            adbc = wk.tile([128, T], f32, tag=f"adbcs_{p}")
            nc.vector.tensor_copy(adbc[:], adbc_ps[:])

            z = wk.tile([128, T], f32, tag=f"z_{p}")
            nc.vector.tensor_tensor(
                z[:].rearrange("p (j c) -> p j c", j=2),
                s["asps"][:].rearrange("p (c j) -> p j c", j=2),
                adbc[:].rearrange("p (j c) -> p j c", j=2), op=Alu.add)
            e = wk.tile([128, T], f32, tag=f"e_{p}")
            nc.vector.scalar_tensor_tensor(
                e[:], z[:], 0.2, z[:], op0=Alu.mult, op1=Alu.max)
            ex = wk.tile([128, T], f32, tag=f"ex_{p}")
            nc.scalar.activation(ex[:], e[:], Act.Exp)
            P = wk.tile([128, T], f32, tag=f"P_{p}")
            nc.vector.tensor_tensor(P[:], ex[:], s["cnt"][:], op=Alu.mult)

            M = wk.tile([128, T * GPT], bf16, tag=f"M_{p}")
            nc.vector.tensor_tensor(
                M[:].rearrange("p (t j) -> p t j", j=GPT),
                P[:].unsqueeze(2).to_broadcast((128, T, GPT)),
                Bmf.unsqueeze(1).to_broadcast((128, T, GPT)),
                op=Alu.mult)
            s["M"] = M

        # ---- aggregation (both branches back to back on PE) ----
        for p in ("u", "d"):
            s = st[p]
            yns = wk.tile([SA, T * GPT], f32, tag=f"yns_{p}")
            s["yns"] = yns
            for h in range(2):
                t0, t1 = (0, T // 2) if h == 0 else (T // 2, T)
                ynT = psY.tile([SA, (t1 - t0) * GPT], f32, tag="ynT",
                               name=f"ynT_{p}{h}")
                for t in range(t0, t1):
                    blk = 0
                    while XG_SPLIT[blk + 1] <= t:
                        blk += 1
                    tt = t - XG_SPLIT[blk]
                    nc.tensor.matmul(
                        ynT[:, GPT * (t - t0):GPT * (t - t0 + 1)],
                        s["xg"][blk][:, SA * tt:SA * tt + SA],
                        s["M"][:, GPT * t:GPT * (t + 1)],
                        start=True, stop=True)
                nc.scalar.activation(
                    yns[:, t0 * GPT:t1 * GPT], ynT[:], Act.Copy)

        # ---- normalize + project + exp per branch ----
        for p in ("u", "d"):
            s = st[p]
            yns = s["yns"]
            dn = wk.tile([1, GPC], f32, tag=f"dn_{p}")
            nc.vector.tensor_scalar(
                dn[:], yns[S:S + 1, 0:GPC], eps, None, op0=Alu.add)
            rp = wk.tile([1, GPC], f32, tag=f"rp_{p}")
            nc.vector.reciprocal_approx_fast(rp[:], dn[:])
            rbc = ps1.tile([S, GPC], f32, tag="mix", name=f"rbc_{p}")
            nc.tensor.matmul(rbc[:], ones64, rp[:], start=True, stop=True)
            ynrm = wk.tile([S, GPC], bf16, tag=f"ynrm_{p}")
            nc.vector.tensor_tensor(ynrm[:], yns[0:S, 0:GPC], rbc[:],
                                    op=Alu.mult)
            hT = ps1.tile([H, GPC], f32, tag="hT", name=f"hT_{p}")
            nc.tensor.matmul(hT[:], Ws[p], ynrm[:], start=True, stop=True)
            # eu = exp(-(h + b)) ;  sigmoid(h+b) = 1/(1+eu)
            eu = wk.tile([H, GPC], bf16, tag=f"eu_{p}")
            nc.scalar.activation(eu[:], hT[:], Act.Exp, bias=nbias[p],
                                 scale=-1.0)
            s["eu"] = eu

        # ---- combine: sg_u*sg_d = 1/((1+eu)(1+ed)) ----
        eu1 = wk.tile([H, GPC], bf16, tag="eu1")
        nc.vector.tensor_scalar(
            eu1[:], st["u"]["eu"][:], 1.0, None, op0=Alu.add)
        ed1 = wk.tile([H, GPC], bf16, tag="ed1")
        nc.vector.tensor_scalar(
            ed1[:], st["d"]["eu"][:], 1.0, None, op0=Alu.add)
        q = wk.tile([H, GPC], f32, tag="q")
        nc.vector.tensor_tensor(q[:], eu1[:], ed1[:], op=Alu.mult)
        r32 = wk.tile([H, GPC], f32, tag="r32")
        nc.vector.reciprocal_approx_fast(r32[:], q[:])
        r = wk.tile([H, GPC], bf16, tag="r")
        nc.vector.tensor_copy(r[:], r32[:])
        o_ps = ps1.tile([1, GPC], f32, tag="mix", name="o_ps")
        nc.tensor.matmul(o_ps[:], mlpW, r[:], start=True, stop=True)
        o_sb = wk.tile([1, GPC], f32, tag="o_sb")
        nc.vector.tensor_scalar(
            o_sb[:], o_ps[:], mlpb, None, op0=Alu.add)
        nc.sync.dma_start(out_dram.ap(), o_sb[:])

    nc.compile()
    return nc


def _get_module(K):
    key = ("nc", K)
    if key not in _CACHE:
        _CACHE[key] = _build_module(K)
    return _CACHE[key]


def _branch_meta(ei):
    """nodes/counts/slots for one branch (host, structure only)."""
    src = np.asarray(ei[0]).astype(np.int64)
    dst = np.asarray(ei[1]).astype(np.int64)
    valid = (dst % NPG) == (NPG - 1)
    nodes, counts = np.unique(src[valid], return_counts=True)
    gids = nodes // NPG
    order = np.argsort(gids, kind="stable")
    gs = gids[order]
    first = np.r_[True, gs[1:] != gs[:-1]]
    idx_of_first = np.maximum.accumulate(
        np.where(first, np.arange(len(gs)), 0))
    slot_sorted = np.arange(len(gs)) - idx_of_first
    slot = np.empty(len(nodes), np.int64)
    slot[order] = slot_sorted
    maxd = int(slot.max()) + 1 if slot.size else 0
    return nodes, counts, gids, slot, maxd


def _prep_branch(x, W, att_src, att_dst, meta, K):
    import ml_dtypes
    bf = ml_dtypes.bfloat16
    GPT, T, NT, XTC, NCH = _layout(K)
    x = np.asarray(x, np.float32)
    W = np.asarray(W, np.float32)
    w_src = (W @ np.asarray(att_src, np.float32)).astype(np.float32)
    w_dst = (W @ np.asarray(att_dst, np.float32)).astype(np.float32)
    nodes, counts, gids, slot, _ = meta

    per_core = []
    for c in range(NC):
        g_lo, g_hi = c * GPC, (c + 1) * GPC
        m = (gids >= g_lo) & (gids < g_hi)
        nl, cl, gl, sl = nodes[m], counts[m], gids[m] - g_lo, slot[m]
        t = gl // GPT
        part = (gl % GPT) * K + sl
        xg = np.zeros((128, T, SA), np.float32)
        xg[part, t, :S] = x[nl]
        xg[:, :, S] = 1.0
        xg2 = np.ascontiguousarray(xg.reshape(128, T * SA)).astype(bf)
        cnt_t = np.zeros((128, T), np.float32)
        cnt_t[part, t] = cl.astype(np.float32)
        xflat = np.zeros((NT, S), np.float32)
        xflat[t * 128 + part] = x[nl]
        xtv = xflat.reshape(2, XTC, S).transpose(0, 2, 1)
        xtv = np.ascontiguousarray(xtv.reshape(128, XTC)).astype(bf)
        lg = np.arange(g_lo * NPG + NPG - 1, g_hi * NPG, NPG)
        xl = x[lg].reshape(GPC, S)
        xl2 = np.zeros((128, GPT, S), np.float32)
        gg = np.arange(GPC)
        xl2[gg // GPT, gg % GPT] = xl
        xl2 = np.ascontiguousarray(xl2.reshape(128, GPT * S)).astype(bf)
        per_core.append({"xg": xg2, "xt": xtv, "cnt": cnt_t, "xl2": xl2})

    shared = {"w_src": w_src, "w_dst": w_dst, "W": W}
    return per_core, shared


def _build_in_maps(inputs, metas, K):
    import ml_dtypes
    bf = ml_dtypes.bfloat16
    GPT, T, NT, XTC, NCH = _layout(K)
    pcs = {}
    shareds = {}
    pcs["u"], shareds["u"] = _prep_branch(
        inputs["up_x"], inputs["up_W"],
        inputs["up_att_src"], inputs["up_att_dst"], metas["u"], K)
    pcs["d"], shareds["d"] = _prep_branch(
        inputs["down_x"], inputs["down_W"],
        inputs["down_att_src"], inputs["down_att_dst"], metas["d"], K)

    pp = np.arange(128)
    cstf = np.zeros((128, 204), np.float32)
    cstf[:, 0:128] = np.eye(128, dtype=np.float32)
    cstf[0, 128] = 1e-16
    cstf[0, 129] = float(np.asarray(inputs["mlp_b"]).reshape(-1)[0])
    cstf[:, 130] = -np.asarray(inputs["up_bias"], np.float32)
    cstf[:, 131] = -np.asarray(inputs["down_bias"], np.float32)
    # Bmf [128, GPT]: 1 if p//K == j (p < GPT*K)
    cstf[pp[:GPT * K], 132 + pp[:GPT * K] // K] = 1.0
    cstf[0, 140:204] = 1.0

    cstb = np.zeros((128, 520), np.float32)
    for i, p in enumerate(("u", "d")):
        ws = shareds[p]["w_src"]
        cstb[0:S, 0 + 2 * i] = ws
        cstb[S:128, 1 + 2 * i] = ws
        cstb[:, 4 + S * i:4 + S * (i + 1)] = np.broadcast_to(
            shareds[p]["w_dst"], (128, S))
    # Qm [GPT, 128]: 1 if m//K == j (m < GPT*K)
    for j in range(GPT):
        cstb[j, 132 + j * K:132 + (j + 1) * K] = 1.0
    cstb[0:S, 260:388] = shareds["u"]["W"]
    cstb[0:S, 388:516] = shareds["d"]["W"]
    cstb[:, 516] = np.asarray(inputs["mlp_W"], np.float32).reshape(H)

    common = {
        "cstf": cstf,
        "cstb": cstb.astype(bf),
    }

    in_maps = []
    for c in range(NC):
        m = dict(common)
        for p in ("u", "d"):
            for k2, v in pcs[p][c].items():
                m[f"{p}_{k2}"] = v
        in_maps.append(m)
    return in_maps


def kernel(**inputs):
    from concourse.bass_utils import run_bass_kernel_spmd

    metas = {"u": _branch_meta(inputs["up_edge_index"]),
             "d": _branch_meta(inputs["down_edge_index"])}
    maxd = max(metas["u"][4], metas["d"][4])
    K = 18 if maxd <= 18 else maxd  # compiled layout adapts to the data
    nc = _get_module(K)
    in_maps = _build_in_maps(inputs, metas, K)
    res = run_bass_kernel_spmd(nc, in_maps, core_ids=list(range(NC)))
    out = np.concatenate(
        [np.asarray(r["out"], np.float32).reshape(GPC) for r in res.results])
    return out.reshape(G, 1)
